# revision 10
# baseline (speedup 1.0000x reference)
import sys

sys.path.insert(0, "/opt/trn_rl_repo")

import numpy as np

import concourse.bass as bass
import concourse.mybir as mybir
from concourse.tile import ScopedClock, TileContext
from concourse.bass_utils import run_bass_kernel_spmd

B, S, D = 4, 4096, 1024
N_CORES = 8
SS = S // N_CORES  # 512 seq rows per core
P = 128
NJ = SS // P  # 4 partition-blocks of seq per core
SCALE = 32.0  # sqrt(1024)

_TRACE = False
_LAST_RESULTS = None


class SplitDrainTileContext(TileContext):
    # The final drain aggregates one wait per live semaphore, but the ISA
    # allows a single sync-wait per instruction. Split into sequential drains.
    def _drain_and_barrier(self, tick_clock, wait_clock):
        drain_inst = self.nc.sync.drain()
        wait_clock.add_sem_waits(
            drain_inst.ins, ScopedClock({None: tick_clock.global_clock})
        )
        waits = list(drain_inst.ins.sync_info.on_wait or [])
        if len(waits) > 1:
            drain_inst.ins.sync_info.on_wait = waits[:1]
            for w in waits[1:]:
                extra = self.nc.sync.drain()
                extra.ins.sync_info = mybir.SyncInfo(on_wait=[w], on_update=[])
        self.nc.all_engine_barrier()
        assert self.sems is not None
        popped = self.nc._tile_sem_poison_stack.pop()
        assert popped is self._sem_poison
        self.nc.clear_and_free_semaphores(list(self.sems.allocated().values()))
        self.nc.all_engine_barrier()


def _make_pe() -> np.ndarray:
    # Match reference f32 arithmetic: angle computed via f32 pow + f32 divide,
    # sin/cos evaluated in f64 on the rounded f32 angle, cast back to f32.
    pos = np.arange(S, dtype=np.float32)[:, None]
    i = np.arange(D, dtype=np.float32)[None, :]
    expnt = (np.float32(2.0) * i / np.float32(D)).astype(np.float32)
    denom = np.power(np.float32(10000.0), expnt).astype(np.float32)
    angle = (pos / denom).astype(np.float32)
    even = (np.arange(D) % 2 == 0)[None, :]
    pe = np.where(even, np.sin(angle.astype(np.float64)), np.cos(angle.astype(np.float64)))
    return pe.astype(np.float32)


def _build() -> bass.Bass:
    nc = bass.Bass()
    f32 = mybir.dt.float32
    emb = nc.declare_dram_parameter("emb", [B, SS, D], f32, isOutput=False)
    pe = nc.declare_dram_parameter("pe", [SS, D], f32, isOutput=False)
    out = nc.declare_dram_parameter("out", [B, SS, D], f32, isOutput=True)

    with SplitDrainTileContext(nc) as tc:
        with tc.tile_pool(name="pool", bufs=1) as pool:
            # pe chunk [512,1024] -> one [128, 4*1024] tile, layout [p, (j d)]
            pe_tile = pool.tile([P, NJ * D], f32, tag="pe")
            nc.sync.dma_start(
                out=pe_tile[:].rearrange("p (j d) -> p j d", d=D),
                in_=pe[:].rearrange("(j p) d -> p j d", p=P),
            )
            # Primer: absorb the pe DMA wait on the DVE engine so the STTs
            # below never need a second sync wait (custom DVE STT opcode
            # supports only one).
            scratch = pool.tile([P, 1], f32, tag="scratch")
            nc.vector.tensor_copy(out=scratch[:], in_=pe_tile[:, 0:1])

            # Two result tiles, each covering 2 batches, so each out-DMA has a
            # single writer (the STTs on DVE) -> exactly 1 sync wait (DVE sem).
            # 7 DMAs total (pe + 4 in + 2 out) <= 8 completion lanes, so no
            # lane-reuse waits either (ISA allows 1 wait per instruction).
            res = [
                pool.tile([P, 2 * NJ * D], f32, tag=f"res{g}", name=f"res{g}")
                for g in range(2)
            ]
            for b in range(B):
                t = pool.tile([P, NJ * D], f32, tag=f"in{b}")
                nc.sync.dma_start(
                    out=t[:].rearrange("p (j d) -> p j d", d=D),
                    in_=emb[b].rearrange("(j p) d -> p j d", p=P),
                )
                g, h = divmod(b, 2)
                nc.vector.scalar_tensor_tensor(
                    out=res[g][:, h * NJ * D : (h + 1) * NJ * D],
                    in0=t[:],
                    scalar=SCALE,
                    in1=pe_tile[:],
                    op0=mybir.AluOpType.mult,
                    op1=mybir.AluOpType.add,
                )
                if h == 1:
                    nc.sync.dma_start(
                        out=out[2 * g : 2 * g + 2].rearrange(
                            "b (j p) d -> p b j d", p=P
                        ),
                        in_=res[g][:].rearrange("p (b j d) -> p b j d", b=2, d=D),
                    )
    return nc


_CACHE: dict = {}


def kernel(embeddings: np.ndarray) -> np.ndarray:
    global _LAST_RESULTS
    emb = np.asarray(embeddings, dtype=np.float32)
    if "nc" not in _CACHE:
        _CACHE["nc"] = _build()
        _CACHE["pe"] = _make_pe()
    nc = _CACHE["nc"]
    pe = _CACHE["pe"]

    in_maps = []
    for c in range(N_CORES):
        sl = emb[:, c * SS : (c + 1) * SS, :]
        in_maps.append(
            {
                "emb": np.ascontiguousarray(sl),
                "pe": np.ascontiguousarray(pe[c * SS : (c + 1) * SS]),
            }
        )
    res = run_bass_kernel_spmd(
        nc, in_maps, core_ids=list(range(N_CORES)), trace=_TRACE
    )
    _LAST_RESULTS = res
    return np.concatenate([r["out"] for r in res.results], axis=1)


# revision 12
# speedup vs baseline: 1.1504x; 1.1504x over previous
import sys

sys.path.insert(0, "/opt/trn_rl_repo")

import numpy as np

import concourse.bacc as bacc
import concourse.bass as bass
import concourse.mybir as mybir
from concourse.tile import TileContext
from concourse.bass_utils import run_bass_kernel_spmd

B, S, D = 4, 4096, 1024
N_CORES = 8
SS = S // N_CORES  # 512 seq rows per core
P = 128
NJ = SS // P  # 4 partition-blocks of seq per core
SCALE = 32.0  # sqrt(1024)

TWO_PI = float(np.float32(2.0 * np.pi))
INV_2PI = float(np.float32(1.0 / (2.0 * np.pi)))
PI_F = float(np.float32(np.pi))

_TRACE = False
_LAST_RESULTS = None


def _make_tab() -> np.ndarray:
    # [1, 2D]: per-column frequency 1/10000^(2i/D) and phase (pi/2 on odd
    # columns turns sin into cos), matching reference f32 arithmetic.
    i = np.arange(D, dtype=np.float32)
    expnt = (np.float32(2.0) * i / np.float32(D)).astype(np.float32)
    denom = np.power(np.float32(10000.0), expnt).astype(np.float32)
    freq = (np.float32(1.0) / denom).astype(np.float32)
    phase = np.where(
        np.arange(D) % 2 == 0, np.float32(0.0), np.float32(np.pi / 2)
    ).astype(np.float32)
    return np.concatenate([freq, phase])[None, :]


def _build() -> bass.Bass:
    nc = bacc.Bacc()
    f32 = mybir.dt.float32
    i32 = mybir.dt.int32
    emb = nc.declare_dram_parameter("emb", [B, SS, D], f32, isOutput=False)
    srow = nc.declare_dram_parameter("srow", [P, 1], f32, isOutput=False)
    tab = nc.declare_dram_parameter("tab", [1, 2 * D], f32, isOutput=False)
    out = nc.declare_dram_parameter("out", [B, SS, D], f32, isOutput=True)

    with TileContext(nc) as tc:
        with tc.tile_pool(name="pool", bufs=2) as pool:
            # ---- compute pe[512,1024] on-chip as [128, 4*1024] ----
            tab0 = pool.tile([1, 2 * D], f32, tag="tab0", bufs=1)
            nc.sync.dma_start(out=tab0[:], in_=tab[:])
            tabb = pool.tile([P, 2 * D], f32, tag="tabb", bufs=1)
            nc.gpsimd.partition_broadcast(tabb[:], tab0[:])

            srow_t = pool.tile([P, 1], f32, tag="srow", bufs=1)
            nc.sync.dma_start(out=srow_t[:], in_=srow[:])
            svs = [srow_t]
            for j in range(1, NJ):
                sv = pool.tile([P, 1], f32, tag=f"sv{j}", name=f"sv{j}", bufs=1)
                nc.vector.tensor_scalar_add(
                    out=sv[:], in0=srow_t[:], scalar1=float(j * P)
                )
                svs.append(sv)

            ang = pool.tile([P, NJ * D], f32, tag="ang", bufs=1)
            q = pool.tile([P, NJ * D], f32, tag="q", bufs=1)
            ki = pool.tile([P, NJ * D], i32, tag="ki", bufs=1)
            kf = pool.tile([P, NJ * D], f32, tag="kf", bufs=1)
            red = pool.tile([P, NJ * D], f32, tag="red", bufs=1)
            pe_tile = pool.tile([P, NJ * D], f32, tag="pe", bufs=1)
            for j in range(NJ):
                sl = slice(j * D, (j + 1) * D)
                nc.vector.scalar_tensor_tensor(
                    out=ang[:, sl], in0=tabb[:, 0:D], scalar=svs[j][:],
                    in1=tabb[:, D : 2 * D],
                    op0=mybir.AluOpType.mult, op1=mybir.AluOpType.add,
                )
            # range-reduce to [-pi, pi]: k = int(ang/2pi + 0.5); red = ang - k*2pi
            nc.vector.tensor_scalar(
                out=q[:], in0=ang[:], scalar1=INV_2PI, scalar2=0.5,
                op0=mybir.AluOpType.mult, op1=mybir.AluOpType.add,
            )
            nc.vector.tensor_copy(out=ki[:], in_=q[:])
            nc.vector.tensor_copy(out=kf[:], in_=ki[:])
            nc.vector.scalar_tensor_tensor(
                out=red[:], in0=kf[:], scalar=-TWO_PI, in1=ang[:],
                op0=mybir.AluOpType.mult, op1=mybir.AluOpType.add,
            )
            nc.vector.add_range_wrap(
                out=red[:], in_=red[:], shift=0.0, bound=PI_F, period=TWO_PI
            )
            nc.scalar.activation(
                out=pe_tile[:], in_=red[:], func=mybir.ActivationFunctionType.Sin
            )

            # ---- out[b] = emb[b]*sqrt(D) + pe ----
            for b in range(B):
                t = pool.tile([P, NJ * D], f32, tag="io", name="t")
                nc.sync.dma_start(
                    out=t[:].rearrange("p (j d) -> p j d", d=D),
                    in_=emb[b].rearrange("(j p) d -> p j d", p=P),
                )
                nc.vector.scalar_tensor_tensor(
                    out=t[:],
                    in0=t[:],
                    scalar=SCALE,
                    in1=pe_tile[:],
                    op0=mybir.AluOpType.mult,
                    op1=mybir.AluOpType.add,
                )
                nc.sync.dma_start(
                    out=out[b].rearrange("(j p) d -> p j d", p=P),
                    in_=t[:].rearrange("p (j d) -> p j d", d=D),
                )
    nc.finalize()
    return nc


_CACHE: dict = {}


def kernel(embeddings: np.ndarray) -> np.ndarray:
    global _LAST_RESULTS
    emb = np.asarray(embeddings, dtype=np.float32)
    if "nc" not in _CACHE:
        _CACHE["nc"] = _build()
        _CACHE["tab"] = _make_tab()
    nc = _CACHE["nc"]
    tab = _CACHE["tab"]

    in_maps = []
    for c in range(N_CORES):
        sl = emb[:, c * SS : (c + 1) * SS, :]
        srow = (np.arange(P, dtype=np.float32) + c * SS)[:, None]
        in_maps.append(
            {
                "emb": np.ascontiguousarray(sl),
                "srow": srow,
                "tab": tab,
            }
        )
    res = run_bass_kernel_spmd(
        nc, in_maps, core_ids=list(range(N_CORES)), trace=_TRACE
    )
    _LAST_RESULTS = res
    return np.concatenate([r["out"] for r in res.results], axis=1)
